# revision 1
# baseline (speedup 1.0000x reference)
"""Trainium2 Bass kernel for nn_ContrastiveLoss (N=384, D=128, 8 cores).

Math restructure (validated exactly against the reference):
  For each anchor row i and positive p (both off-diagonal), with
    a[i,j] = |y_i - y_j|,  w[i,j] = exp(-dist(z_i,z_j)/TEMP) * sigmoid(TAU*a[i,j]),
    u = w * [y_j > y_i] * [j != i],  v = w * [y_j <= y_i] * [j != i],
    S1[i,p] = sum_j u[i,j] * [a[i,j] < a[i,p]],  S0 likewise with v,
    T1 = sum_j u,  T0 = sum_j v:
  denom[i,p] = (POS_W-1)*S1 - NEG_W*S0 + NEG_W*T0 + T1
  loss = -(sum_{i,p!=i} s[i,p] - sum_{i,p!=i} log denom[i,p]) / (N*(N-1)),
  s = -dist/TEMP.  (The reference's row-max shift is exactly 0, so it's skipped.)

Per core (48 rows): the comparison tile C'[j,p] = [a_j < a_p] is built on the
Vector engine (one tensor_scalar is_gt per 128-j chunk) and contracted on the
TensorEngine with lhsT = [u_col, v_col] (M=2), accumulating S1/S0 in PSUM.
"""

import os
import sys

import numpy as np

for _p in ("/opt/trn_rl_repo", "/root/.axon_site/_ro/trn_rl_repo"):
    if os.path.isdir(_p) and _p not in sys.path:
        sys.path.insert(0, _p)

import concourse.bass as bass
import concourse.bacc as bacc
import concourse.mybir as mybir
from concourse import tile
from concourse.bass_utils import run_bass_kernel_spmd

F32 = mybir.dt.float32
AF = mybir.ActivationFunctionType
OP = mybir.AluOpType

B = 192          # batch
N = 2 * B        # 384 rows/cols of the pairwise matrices
D = 128          # embedding dim
NC = 8           # cores
R = N // NC      # 48 rows per core
CH = N // 128    # 3 chunks of the j dimension
PW = 920         # packed input width (919 used + 1 pad)

TEMP = 2.0
TAU = 1.0
POS_W = 0.1
NEG_W = 1.0


def _build_program():
    nc = bacc.Bacc("TRN2", target_bir_lowering=False, debug=False, num_devices=NC)

    # ---- I/O (f32). Everything arrives in ONE packed [128, PW] tensor so a
    # single DMA (one queue semaphore) feeds all consumers — walrus rejects
    # compute instructions carrying more than one DMA-queue sync wait.
    # Columns: 0:384 zT | 384:432 zTown | 432:480 yownrep | 480:528 ownidxrep
    #          528:531 ycolc | 531:534 jcolc | 534:918 yrep48 (rows 0:48)
    #          918:919 yowncol (rows 0:48)
    packed = nc.dram_tensor("packed", [128, PW], F32, kind="ExternalInput").ap()
    out = nc.dram_tensor("out", [2, R], F32, kind="ExternalOutput").ap()

    with tile.TileContext(nc) as tc:
        with (
            tc.tile_pool(name="big", bufs=1) as big,
            tc.tile_pool(name="small", bufs=1) as small,
            tc.tile_pool(name="chunk", bufs=3) as chunk,
            tc.tile_pool(name="arep", bufs=4) as arep_pool,
            tc.tile_pool(name="cmp", bufs=18) as cmp_pool,
            tc.tile_pool(name="ps_ss", bufs=1, space="PSUM") as ps_ss,
            tc.tile_pool(name="ps_pre", bufs=1, space="PSUM") as ps_pre,
            tc.tile_pool(name="ps_gt", bufs=3, space="PSUM") as ps_gt,
            tc.tile_pool(name="ps_acc", bufs=1, space="PSUM") as ps_acc,
            tc.tile_pool(name="ps_arep", bufs=2, space="PSUM") as ps_arep,
            tc.tile_pool(name="dram", bufs=1, space="DRAM") as dram_pool,
        ):
            # ---------- load inputs (ONE DMA) ----------
            pk = big.tile([128, PW], F32, tag="packed")
            nc.sync.dma_start(pk[:], packed)
            zT_s = pk[:, 0:N]
            zTown_s = pk[:, N : N + R]
            yownrep = pk[:, N + R : N + 2 * R]
            ownidxrep = pk[:, N + 2 * R : N + 3 * R]
            ycolc = pk[:, N + 3 * R : N + 3 * R + CH]
            jcolc = pk[:, N + 3 * R + CH : N + 3 * R + 2 * CH]
            yrep48 = pk[0:R, N + 3 * R + 2 * CH : 2 * N + 3 * R + 2 * CH]
            yowncol_s = pk[0:R, 2 * N + 3 * R + 2 * CH : 2 * N + 3 * R + 2 * CH + 1]

            ones128 = small.tile([128, 1], F32, tag="ones128")
            nc.vector.memset(ones128[:], 1.0)
            onesrow = small.tile([1, 128], F32, tag="onesrow")
            nc.vector.memset(onesrow[:], 1.0)

            # ---------- A row-block: a[i, p] = |y_p - y_i|  (exact on 2^-23 grid)
            a48raw = big.tile([R, N], F32, tag="a48raw")
            nc.vector.tensor_tensor(
                a48raw[:], yrep48, yowncol_s.to_broadcast((R, N)), op=OP.subtract
            )
            a48 = big.tile([R, N], F32, tag="a48")
            nc.scalar.activation(a48[:], a48raw[:], AF.Abs)

            # ---------- squared norms ----------
            zsq = big.tile([D, N], F32, tag="zsq")
            nc.vector.tensor_tensor(zsq[:], zT_s, zT_s, op=OP.mult)
            zsqown = small.tile([D, R], F32, tag="zsqown")
            nc.vector.tensor_tensor(zsqown[:], zTown_s, zTown_s, op=OP.mult)

            n2own_ps = ps_pre.tile([1, R], F32, tag="pre")
            nc.tensor.matmul(n2own_ps[:], ones128[:], zsqown[:], start=True, stop=True)
            n2own_s = small.tile([1, R], F32, tag="n2own_s")
            nc.vector.tensor_copy(n2own_s[:], n2own_ps[:])
            n2ownrep_ps = ps_pre.tile([128, R], F32, tag="pre")
            nc.tensor.matmul(n2ownrep_ps[:], onesrow[:], n2own_s[:], start=True, stop=True)
            n2ownrep = small.tile([128, R], F32, tag="n2ownrep")
            nc.vector.tensor_copy(n2ownrep[:], n2ownrep_ps[:])

            n2colc = small.tile([128, CH], F32, tag="n2colc")
            for c in range(CH):
                n2c_ps = ps_pre.tile([128, 1], F32, tag="pre")
                nc.tensor.matmul(
                    n2c_ps[:],
                    zsq[:, c * 128 : (c + 1) * 128],
                    ones128[:],
                    start=True,
                    stop=True,
                )
                nc.vector.tensor_copy(n2colc[:, c : c + 1], n2c_ps[:])

            # ---------- transposed-side prep per chunk ----------
            atc = small.tile([128, CH * R], F32, tag="atc")       # |y_j - y_i|
            uvt = small.tile([128, CH * 2 * R], F32, tag="uvt")   # interleaved u,v cols
            cs_ps = ps_acc.tile([1, 2 * R], F32, tag="acc")        # [sum_j w_off | sum_j dist_off]
            for c in range(CH):
                csl = slice(c * R, (c + 1) * R)
                atcraw = chunk.tile([128, R], F32, tag="atcraw")
                nc.vector.tensor_tensor(
                    atcraw[:],
                    yownrep,
                    ycolc[:, c : c + 1].to_broadcast((128, R)),
                    op=OP.subtract,
                )
                nc.scalar.activation(atc[:, csl], atcraw[:], AF.Abs)

                samet = chunk.tile([128, R], F32, tag="samet")
                nc.vector.tensor_tensor(
                    samet[:],
                    yownrep,
                    ycolc[:, c : c + 1].to_broadcast((128, R)),
                    op=OP.is_lt,
                )
                ndt = chunk.tile([128, R], F32, tag="ndt")
                nc.vector.tensor_tensor(
                    ndt[:],
                    ownidxrep,
                    jcolc[:, c : c + 1].to_broadcast((128, R)),
                    op=OP.not_equal,
                )

                gt_ps = ps_gt.tile([128, R], F32, tag="gt")
                nc.tensor.matmul(
                    gt_ps[:],
                    zT_s[:, c * 128 : (c + 1) * 128],
                    zTown_s,
                    start=True,
                    stop=True,
                )
                sqt = chunk.tile([128, R], F32, tag="sqt")
                # sq = n2own + n2col - 2*G
                nc.vector.tensor_scalar(sqt[:], gt_ps[:], -2.0, None, op0=OP.mult)
                nc.vector.tensor_tensor(sqt[:], sqt[:], n2ownrep[:], op=OP.add)
                nc.vector.tensor_tensor(
                    sqt[:], sqt[:], n2colc[:, c : c + 1].to_broadcast((128, R)), op=OP.add
                )
                sqr = chunk.tile([128, R], F32, tag="sqr")
                nc.scalar.activation(sqr[:], sqt[:], AF.Relu)
                distt = chunk.tile([128, R], F32, tag="distt")
                nc.scalar.activation(distt[:], sqr[:], AF.Sqrt)
                et = chunk.tile([128, R], F32, tag="et")
                nc.scalar.activation(et[:], distt[:], AF.Exp, scale=-1.0 / TEMP)
                dwt = chunk.tile([128, R], F32, tag="dwt")
                nc.scalar.activation(dwt[:], atc[:, csl], AF.Sigmoid, scale=TAU)

                # wd = [w*offdiag | dist*offdiag]  (one tile so one PE colsum matmul)
                wd = chunk.tile([128, 2 * R], F32, tag="wd")
                wt = chunk.tile([128, R], F32, tag="wt")
                nc.vector.tensor_tensor(wt[:], et[:], dwt[:], op=OP.mult)
                nc.vector.tensor_tensor(wd[:, 0:R], wt[:], ndt[:], op=OP.mult)
                nc.vector.tensor_tensor(wd[:, R : 2 * R], distt[:], ndt[:], op=OP.mult)

                # interleaved u,v columns for the main-loop lhsT
                base = c * 2 * R
                uv_u = uvt[:, base : base + 2 * R : 2]
                uv_v = uvt[:, base + 1 : base + 2 * R : 2]
                nc.vector.tensor_tensor(uv_u, wd[:, 0:R], samet[:], op=OP.mult)
                nc.vector.tensor_tensor(uv_v, wd[:, 0:R], uv_u, op=OP.subtract)

                nc.tensor.matmul(
                    cs_ps[:], ones128[:], wd[:], start=(c == 0), stop=(c == CH - 1)
                )

            cs_s = small.tile([1, 2 * R], F32, tag="cs_s")
            nc.vector.tensor_copy(cs_s[:], cs_ps[:])
            # cs_s[0, 0:R] = c_i = T0+T1 ;  cs_s[0, R:2R] = sum_{p!=i} dist[i,p]
            crep_ps = ps_pre.tile([128, R], F32, tag="pre")
            nc.tensor.matmul(crep_ps[:], onesrow[:], cs_s[0:1, 0:R], start=True, stop=True)
            crep48 = small.tile([128, R], F32, tag="crep48")
            nc.vector.tensor_copy(crep48[:], crep_ps[:])

            # ---------- main loop ----------
            # a48 rows flattened into partition 0 so the per-row PE outer
            # product (ones ⊗ a-row) can read its rhs at partition base 0.
            arowflat = small.tile([1, R * N], F32, tag="arowflat")
            nc.sync.dma_start(
                arowflat[0:1, :].rearrange("a (p f) -> a p f", p=R, f=N), a48[:]
            )
            # Transposed outputs: for row i, chunk-of-p psub, S1/S0 land in
            # sst[:, psub*2R + 2i + {0,1}] (partition = p within psub).
            sst_ps = ps_ss.tile([128, CH * 2 * R], F32, tag="sst")
            for i in range(R):
                arep_ps = ps_arep.tile([128, N], F32, tag="arep_ps")
                nc.tensor.matmul(
                    arep_ps[:],
                    onesrow[:],
                    arowflat[0:1, i * N : (i + 1) * N],
                    start=True,
                    stop=True,
                )
                arep = arep_pool.tile([128, N], F32, tag="arep")
                nc.vector.tensor_copy(arep[:], arep_ps[:])
                for c in range(CH):
                    cp = cmp_pool.tile([128, N], F32, tag="cp")
                    nc.vector.tensor_scalar(
                        cp[:],
                        arep[:],
                        atc[:, c * R + i : c * R + i + 1],
                        None,
                        op0=OP.is_gt,
                    )
                    for ps in range(CH):
                        # One accumulation group spans the whole bank: only the
                        # very first matmul starts it (start=True pending-zeroes
                        # the full 2KB zero region); per-byte has_written bits
                        # make each sub-region's first write an overwrite.
                        nc.tensor.matmul(
                            sst_ps[:, ps * 2 * R + 2 * i : ps * 2 * R + 2 * i + 2],
                            cp[:, ps * 128 : (ps + 1) * 128],
                            uvt[:, c * 2 * R + 2 * i : c * 2 * R + 2 * i + 2],
                            start=(i == 0 and c == 0 and ps == 0),
                            stop=(i == R - 1 and c == CH - 1 and ps == CH - 1),
                            skip_group_check=True,
                        )
            sst = small.tile([128, CH * 2 * R], F32, tag="sst_sb")
            nc.vector.tensor_copy(sst[:], sst_ps[:])

            # ---------- postprocess (transposed layout) ----------
            # dent[p_local, ps*R+i] = den[i, ps*128+p_local]
            dent = small.tile([128, CH * R], F32, tag="dent")
            nc.vector.tensor_scalar(
                dent[:], sst[:, 0 : CH * 2 * R : 2], POS_W - 1.0, None, op0=OP.mult
            )
            nc.vector.tensor_tensor(
                dent[:], dent[:], sst[:, 1 : CH * 2 * R : 2], op=OP.subtract
            )
            for c in range(CH):
                nc.vector.tensor_tensor(
                    dent[:, c * R : (c + 1) * R],
                    dent[:, c * R : (c + 1) * R],
                    crep48[:],
                    op=OP.add,
                )
            lnt = small.tile([128, CH * R], F32, tag="lnt")
            nc.scalar.activation(lnt[:], dent[:], AF.Ln)
            lds_ps = ps_acc.tile([1, CH * R], F32, tag="acc")
            nc.tensor.matmul(lds_ps[:], ones128[:], lnt[:], start=True, stop=True)
            lds = small.tile([1, CH * R], F32, tag="lds_s")
            nc.vector.tensor_copy(lds[:], lds_ps[:])

            # combine psub partials; subtract ln(c_i) for the excluded p=i column
            lnc = small.tile([1, R], F32, tag="lnc")
            nc.scalar.activation(lnc[:], cs_s[0:1, 0:R], AF.Ln)
            lnc2 = small.tile([1, R], F32, tag="lnc2")
            nc.vector.tensor_copy(lnc2[:], lnc[:])
            acc = small.tile([1, R], F32, tag="acc")
            nc.vector.tensor_tensor(acc[:], lds[0:1, 0:R], lds[0:1, R : 2 * R], op=OP.add)
            nc.vector.tensor_tensor(acc[:], acc[:], lds[0:1, 2 * R : 3 * R], op=OP.add)
            logd_t = small.tile([1, R], F32, tag="logd_t")
            nc.vector.tensor_tensor(logd_t[:], acc[:], lnc2[:], op=OP.subtract)
            # row0 = sum_{p!=i} s[i,p] = -dist_off_rowsum / TEMP
            ssum_t = small.tile([1, R], F32, tag="ssum_t")
            nc.scalar.activation(
                ssum_t[:], cs_s[0:1, R : 2 * R], AF.Copy, scale=-1.0 / TEMP
            )
            nc.sync.dma_start(out[0:1, :], ssum_t[:])
            nc.sync.dma_start(out[1:2, :], logd_t[:])

    nc.compile()
    return nc


_NC_CACHE = None


def _get_nc():
    global _NC_CACHE
    if _NC_CACHE is None:
        _NC_CACHE = _build_program()
    return _NC_CACHE


def _make_in_maps(embeddings, targets):
    emb = np.ascontiguousarray(np.asarray(embeddings, dtype=np.float32))
    tgt = np.ascontiguousarray(np.asarray(targets, dtype=np.float32))
    z = emb.transpose(1, 0, 2).reshape(N, D)
    zT = np.ascontiguousarray(z.T)                       # [D, N]
    y = np.concatenate([tgt, tgt], axis=0)[:, 0]         # [N]
    jidx = np.arange(N, dtype=np.float32)
    in_maps = []
    for core in range(NC):
        sl = slice(core * R, (core + 1) * R)
        p = np.zeros((128, PW), np.float32)
        p[:, 0:N] = zT
        p[:, N : N + R] = zT[:, sl]
        p[:, N + R : N + 2 * R] = y[None, sl]                       # yownrep
        p[:, N + 2 * R : N + 3 * R] = jidx[None, sl]                # ownidxrep
        p[:, N + 3 * R : N + 3 * R + CH] = y.reshape(CH, 128).T     # ycolc
        p[:, N + 3 * R + CH : N + 3 * R + 2 * CH] = jidx.reshape(CH, 128).T
        p[0:R, N + 3 * R + 2 * CH : 2 * N + 3 * R + 2 * CH] = y[None, :]  # yrep48
        p[0:R, 2 * N + 3 * R + 2 * CH] = y[sl]                      # yowncol
        in_maps.append({"packed": p})
    return in_maps


def _reduce_outs(outs_list):
    tot_s = 0.0
    tot_logd = 0.0
    for o in outs_list:
        o = np.asarray(o, dtype=np.float64)
        tot_s += o[0, :].sum()
        tot_logd += o[1, :].sum()
    loss = -(tot_s - tot_logd) / (N * (N - 1))
    return np.float32(loss)


def _run(embeddings, targets, trace=False, **kw):
    nc = _get_nc()
    in_maps = _make_in_maps(embeddings, targets)
    res = run_bass_kernel_spmd(nc, in_maps, list(range(NC)), trace=trace, **kw)
    outs = [res.results[c]["out"] for c in range(NC)]
    return _reduce_outs(outs), res


def kernel(embeddings, targets):
    loss, _ = _run(embeddings, targets, trace=False)
    return loss

